# revision 5
# baseline (speedup 1.0000x reference)
"""Multi-head attention (softmax over the QUERY axis) on 8 TRN2 NeuronCores.

Sharding: 2 batches x 4 head-groups (4 heads each) -> 8 cores.
Each core computes, for its (batch b, heads 4g..4g+3):
    qkT = W_{q,k} @ x_b^T + b_{q,k}   [512, 2048]   (e_out on partitions)
    V   = x_b @ W_v^T + b_v           [2048, 256]
    S'  = K Q^T (scores TRANSPOSED)   [k, q] per head
    P   = exp(S'/8) with fused row-sum -> denom[k]  (softmax over q == free dim)
    outT= sum_k (V[k,:]/denom[k]) P[k,:]            [d, q] per head
    part= outT^T @ WoT_g              [2048, 1024]  (partial for this head group)
Host sums the 4 partials per batch and adds bo (the tensor-parallel epilogue).

All matmuls run in float32r (full-rate fp32 on the PE). f32r operands must be
produced by a rounding compute op, so DMA'd tensors pass through a staging
tile + DVE rounding copy; op-produced tiles (qkT via ACT, exp via ACT, V' via
DVE) are written as f32r directly.
"""

import sys

if "/opt/trn_rl_repo" not in sys.path:
    sys.path.insert(0, "/opt/trn_rl_repo")

import numpy as np

import concourse.bass as bass
import concourse.mybir as mybir
import concourse.tile as tile
from concourse import bacc
from concourse.bass_utils import run_bass_kernel_spmd

F32 = mybir.dt.float32
F32R = mybir.dt.float32r
AF = mybir.ActivationFunctionType

B, S, E, H = 2, 2048, 1024, 16
HL = 4  # heads per core
DH = 64
QK = 512  # q+k out dims per core (2*HL*DH)
V3 = 768  # q+k+v out dims per core
NCORES = 8

ET = E // 128  # 8 e-tiles
ST = S // 128  # 16 s-tiles
SC = S // 512  # 4 s/q chunks of 512
KT = ST  # 16 k-tiles

LAST_RESULTS = None


def build_kernel():
    nc = bacc.Bacc("TRN2", target_bir_lowering=False, debug=False, num_devices=NCORES)

    xT = nc.dram_tensor("xT", [E, S], F32, kind="ExternalInput")
    wT = nc.dram_tensor("wT", [E, V3], F32, kind="ExternalInput")
    bq = nc.dram_tensor("bq", [128, 4], F32, kind="ExternalInput")
    bv = nc.dram_tensor("bv", [1, 256], F32, kind="ExternalInput")
    woT = nc.dram_tensor("woT", [2 * 128, E], F32, kind="ExternalInput")
    out = nc.dram_tensor("out", [S, E], F32, kind="ExternalOutput")

    with tile.TileContext(nc) as tc:
        with (
            tc.tile_pool(name="persist", bufs=1) as persist,
            tc.tile_pool(name="smalls", bufs=3) as smalls,
        ):
            qk_sb = persist.tile([128, 4, S], F32R, tag="qk")
            v_sb = persist.tile([128, ST, 256], F32, tag="v")
            outT_sb = persist.tile([128, 2, S], F32, tag="outT")
            outT_r = persist.tile([128, 2, S], F32R, tag="outT_r")
            bq_sb = persist.tile([128, 4], F32, tag="bq")
            bv_sb = persist.tile([1, 256], F32, tag="bv")
            ones_sb = persist.tile([1, 512], F32, tag="ones")
            ones_r = persist.tile([1, 512], F32R, tag="ones_r")
            bv_r = persist.tile([1, 256], F32R, tag="bv_r")

            nc.sync.dma_start(bq_sb[:], bq[:])
            nc.sync.dma_start(bv_sb[:], bv[:])
            nc.vector.memset(ones_sb[:], 1.0)
            nc.vector.tensor_copy(ones_r[:], ones_sb[:])
            nc.vector.tensor_copy(bv_r[:], bv_sb[:])

            # ------- phase 1..3: load+round x/w; V; then qkT --------------
            with (
                tc.tile_pool(name="xw", bufs=1) as xw,
                tc.tile_pool(name="stage", bufs=2) as stage,
                tc.tile_pool(name="qk_ps", bufs=2, space="PSUM") as qk_ps,
                tc.tile_pool(name="v_ps", bufs=2, space="PSUM") as v_ps,
            ):
                xt_sb = xw.tile([128, ET, S], F32R, tag="xt")
                wt_sb = xw.tile([128, ET, V3], F32R, tag="wt")
                for et in range(ET):
                    st_t = stage.tile([128, S], F32, tag="xstage")
                    nc.sync.dma_start(st_t[:], xT[et * 128 : (et + 1) * 128, :])
                    nc.vector.tensor_copy(xt_sb[:, et, :], st_t[:])
                for et in range(ET):
                    st_t = stage.tile([128, V3], F32, tag="wstage")
                    nc.sync.dma_start(st_t[:], wT[et * 128 : (et + 1) * 128, :])
                    nc.vector.tensor_copy(wt_sb[:, et, :], st_t[:])

                # V = x @ Wv^T + bv : per s-tile [128, 256]
                for st in range(ST):
                    pt = v_ps.tile([128, 256], F32, tag="vps")
                    for et in range(ET):
                        nc.tensor.matmul(
                            pt[:],
                            xt_sb[:, et, st * 128 : (st + 1) * 128],
                            wt_sb[:, et, QK:V3],
                            start=(et == 0),
                            stop=False,
                        )
                    nc.tensor.matmul(  # + ones^T bv (bias row)
                        pt[:],
                        ones_r[0:1, 0:128],
                        bv_r[0:1, :],
                        start=False,
                        stop=True,
                    )
                    nc.vector.tensor_copy(v_sb[:, st, :], pt[:])

                # qkT; eo order so pair0's tiles (0: Q01, 2: K01) finish first
                for eo in (0, 2, 1, 3):
                    for sc in range(SC):
                        pt = qk_ps.tile([128, 512], F32, tag="qkps")
                        for et in range(ET):
                            nc.tensor.matmul(
                                pt[:],
                                wt_sb[:, et, eo * 128 : (eo + 1) * 128],
                                xt_sb[:, et, sc * 512 : (sc + 1) * 512],
                                start=(et == 0),
                                stop=(et == ET - 1),
                            )
                        # psum->sbuf on ACT with per-partition bias, rounds to f32r
                        nc.scalar.activation(
                            qk_sb[:, eo, sc * 512 : (sc + 1) * 512],
                            pt[:],
                            AF.Identity,
                            bias=bq_sb[:, eo : eo + 1],
                            scale=1.0,
                        )

            # ------- phase 4: attention per head pair ---------------------
            # S'[k,q] per head via PE; exp+rowsum on ACT; V'=V/denom on DVE;
            # outT accumulated in a [64,2048] PSUM tile per (head, 4-ktile
            # group), flushed into an SBUF accumulator (f32r dst at partition
            # 64 is illegal, so no col-pairing).
            FG = 4  # k-tiles per (c)-accumulation group
            with (
                tc.tile_pool(name="expp", bufs=FG + 1) as expp,
                tc.tile_pool(name="vsp", bufs=FG + 2) as vsp,
                tc.tile_pool(name="sp_ps", bufs=2, space="PSUM") as sp_ps,
                tc.tile_pool(name="ot_ps", bufs=1, space="PSUM") as ot_ps,
            ):
                for p in range(2):  # head pairs (local heads 2p, 2p+1)
                    exs = {}
                    vss = {}
                    for g in range(KT // FG):
                        for kt in range(FG * g, FG * (g + 1)):
                            ex = expp.tile([128, 2, S], F32R, tag="exp")
                            exs[kt] = ex
                            den = smalls.tile([128, 2, 2], F32, tag="den")
                            for half in range(2):
                                for hh in range(2):
                                    sp = sp_ps.tile([128, 1024], F32, tag="sp")
                                    for qc in range(2):
                                        q0 = half * 1024 + qc * 512
                                        nc.tensor.matmul(
                                            sp[:, qc * 512 : (qc + 1) * 512],
                                            qk_sb[
                                                hh * 64 : (hh + 1) * 64,
                                                2 + p,
                                                kt * 128 : (kt + 1) * 128,
                                            ],
                                            qk_sb[
                                                hh * 64 : (hh + 1) * 64, p, q0 : q0 + 512
                                            ],
                                            start=True,
                                            stop=True,
                                        )
                                    nc.scalar.activation(
                                        ex[:, hh, half * 1024 : (half + 1) * 1024],
                                        sp[:],
                                        AF.Exp,
                                        scale=0.125,
                                        accum_out=den[:, hh, half : half + 1],
                                    )
                            dsum = smalls.tile([128, 2], F32, tag="dsum")
                            nc.vector.tensor_add(dsum[:], den[:, :, 0], den[:, :, 1])
                            rec = smalls.tile([128, 2], F32, tag="rec")
                            nc.vector.reciprocal(rec[:], dsum[:])
                            vs = vsp.tile([128, 2, DH], F32R, tag="vs")
                            vss[kt] = vs
                            for hh in range(2):
                                nc.vector.tensor_scalar_mul(
                                    vs[:, hh, :],
                                    in0=v_sb[
                                        :, kt, (2 * p + hh) * 64 : (2 * p + hh + 1) * 64
                                    ],
                                    scalar1=rec[:, hh : hh + 1],
                                )
                        for hh in range(2):
                            oTt = ot_ps.tile([64, S], F32, tag="otps")
                            for j in range(FG):
                                kt = FG * g + j
                                for qc in range(SC):
                                    nc.tensor.matmul(
                                        oTt[:, qc * 512 : (qc + 1) * 512],
                                        vss[kt][:, hh, :],
                                        exs[kt][:, hh, qc * 512 : (qc + 1) * 512],
                                        start=(j == 0),
                                        stop=(j == FG - 1),
                                    )
                            dst = outT_sb[hh * 64 : (hh + 1) * 64, p, :]
                            if g == 0:
                                nc.vector.tensor_copy(dst, oTt[:])
                            else:
                                nc.vector.tensor_add(dst, dst, oTt[:])
                    nc.vector.tensor_copy(
                        outT_r[:, p, :], outT_sb[:, p, :]
                    )

            # ------- phase 5: partial = outT^T @ WoT ----------------------
            with (
                tc.tile_pool(name="wo_out", bufs=2) as wo_out,
                tc.tile_pool(name="f_ps", bufs=2, space="PSUM") as f_ps,
            ):
                wo_sb = wo_out.tile([128, 2, E], F32R, tag="wo")
                for p in range(2):
                    st_t = wo_out.tile([128, E], F32, tag="wostage")
                    nc.sync.dma_start(st_t[:], woT[p * 128 : (p + 1) * 128, :])
                    nc.vector.tensor_copy(wo_sb[:, p, :], st_t[:])
                for st in range(ST):
                    ot = wo_out.tile([128, E], F32, tag="fout")
                    for nck in range(2):
                        pt = f_ps.tile([128, 512], F32, tag="fps")
                        for p in range(2):
                            nc.tensor.matmul(
                                pt[:],
                                outT_r[:, p, st * 128 : (st + 1) * 128],
                                wo_sb[:, p, nck * 512 : (nck + 1) * 512],
                                start=(p == 0),
                                stop=(p == 1),
                            )
                        nc.vector.tensor_copy(ot[:, nck * 512 : (nck + 1) * 512], pt[:])
                    nc.sync.dma_start(out[st * 128 : (st + 1) * 128, :], ot[:])

    nc.compile()
    return nc


def _shard_inputs(input, Wqkv, bqkv, Wo):
    """Build the 8 per-core input dicts (host-side layout/sharding)."""
    in_maps = []
    for c in range(NCORES):
        b = c // 4
        g = c % 4
        heads = range(4 * g, 4 * g + 4)
        rows = (
            [slice(64 * h, 64 * h + 64) for h in heads]
            + [slice(E + 64 * h, E + 64 * h + 64) for h in heads]
            + [slice(2 * E + 64 * h, 2 * E + 64 * h + 64) for h in heads]
        )
        W_sel = np.concatenate([Wqkv[s] for s in rows], axis=0)  # [768, 1024]
        b_sel = np.concatenate([bqkv[s] for s in rows], axis=0)  # [768]
        in_maps.append(
            {
                "xT": np.ascontiguousarray(input[b].T),
                "wT": np.ascontiguousarray(W_sel.T),
                "bq": np.ascontiguousarray(b_sel[:QK].reshape(4, 128).T),
                "bv": np.ascontiguousarray(b_sel[QK:V3].reshape(1, 256)),
                "woT": np.ascontiguousarray(Wo[:, 4 * g * DH : 4 * (g + 1) * DH].T),
            }
        )
    return in_maps


def kernel(input, Wqkv, bqkv, Wo, bo, _trace=False):
    global LAST_RESULTS
    input = np.asarray(input, dtype=np.float32)
    Wqkv = np.asarray(Wqkv, dtype=np.float32)
    bqkv = np.asarray(bqkv, dtype=np.float32)
    Wo = np.asarray(Wo, dtype=np.float32)
    bo = np.asarray(bo, dtype=np.float32)

    nc = build_kernel()
    in_maps = _shard_inputs(input, Wqkv, bqkv, Wo)
    kwargs = {}
    if _trace:
        kwargs = dict(trace=True, trace_cores=[0])
    res = run_bass_kernel_spmd(nc, in_maps, core_ids=list(range(NCORES)), **kwargs)
    LAST_RESULTS = res

    out = np.zeros((B, S, E), dtype=np.float32)
    for c in range(NCORES):
        out[c // 4] += res.results[c]["out"]
    out += bo
    return out


# revision 6
# speedup vs baseline: 1.2511x; 1.2511x over previous
"""Multi-head attention (softmax over the QUERY axis) on 8 TRN2 NeuronCores.

Sharding: 2 batches x 4 head-groups (4 heads each) -> 8 cores.
Each core computes, for its (batch b, heads 4g..4g+3):
    qkT = W_{q,k} @ x_b^T + b_{q,k}   [512, 2048]   (e_out on partitions)
    V   = x_b @ W_v^T + b_v           [2048, 256]
    S'  = K Q^T (scores TRANSPOSED)   [k, q] per head
    P   = exp(S'/8) with fused row-sum -> denom[k]  (softmax over q == free dim)
    outT= sum_k (V[k,:]/denom[k]) P[k,:]            [d, q] per head
    part= outT^T @ WoT_g              [2048, 1024]  (partial for this head group)
Host sums the 4 partials per batch and adds bo (the tensor-parallel epilogue).

Matmul inputs are bf16 (PSUM accumulation stays fp32; softmax statistics and
the V'/denominator math stay fp32). Host pre-casts x/W to bf16, which also
halves the input DMA traffic. Head pairs share the PE array: the scores
matmuls of the two heads use disjoint row groups (d at partitions 0-63 vs
64-127), the attn.V matmuls use disjoint column groups (outT partitions 0-63
vs 64-127), so both run concurrently on the 128x128 array.
"""

import sys

if "/opt/trn_rl_repo" not in sys.path:
    sys.path.insert(0, "/opt/trn_rl_repo")

import numpy as np
import ml_dtypes

import concourse.bass as bass
import concourse.mybir as mybir
import concourse.tile as tile
from concourse import bacc
from concourse.bass_utils import run_bass_kernel_spmd

F32 = mybir.dt.float32
BF16 = mybir.dt.bfloat16
AF = mybir.ActivationFunctionType

B, S, E, H = 2, 2048, 1024, 16
HL = 4  # heads per core
DH = 64
QK = 512  # q+k out dims per core (2*HL*DH)
V3 = 768  # q+k+v out dims per core
NCORES = 8

ET = E // 128  # 8 e-tiles
ST = S // 128  # 16 s-tiles
SC = S // 512  # 4 s/q chunks of 512
KT = ST  # 16 k-tiles

LAST_RESULTS = None


def build_kernel():
    nc = bacc.Bacc("TRN2", target_bir_lowering=False, debug=False, num_devices=NCORES)

    xT = nc.dram_tensor("xT", [E, S], BF16, kind="ExternalInput")
    wT = nc.dram_tensor("wT", [E, V3], BF16, kind="ExternalInput")
    bq = nc.dram_tensor("bq", [128, 4], F32, kind="ExternalInput")
    bv = nc.dram_tensor("bv", [1, 256], BF16, kind="ExternalInput")
    woT = nc.dram_tensor("woT", [2 * 128, E], BF16, kind="ExternalInput")
    out = nc.dram_tensor("out", [S, E], F32, kind="ExternalOutput")

    with tile.TileContext(nc) as tc:
        with (
            tc.tile_pool(name="persist", bufs=1) as persist,
            tc.tile_pool(name="smalls", bufs=3) as smalls,
        ):
            qk_sb = persist.tile([128, 4, S], BF16, tag="qk")
            v_sb = persist.tile([128, ST, 256], F32, tag="v")
            outT_sb = persist.tile([128, 2, S], BF16, tag="outT")
            bq_sb = persist.tile([128, 4], F32, tag="bq")
            bv_sb = persist.tile([1, 256], BF16, tag="bv")
            ones_sb = persist.tile([1, 512], BF16, tag="ones")

            nc.sync.dma_start(bq_sb[:], bq[:])
            nc.sync.dma_start(bv_sb[:], bv[:])
            nc.vector.memset(ones_sb[:], 1.0)

            # ------- phase 1..3: load x/w; V; then qkT --------------------
            with (
                tc.tile_pool(name="xw", bufs=1) as xw,
                tc.tile_pool(name="qk_ps", bufs=2, space="PSUM") as qk_ps,
                tc.tile_pool(name="v_ps", bufs=2, space="PSUM") as v_ps,
            ):
                xt_sb = xw.tile([128, ET, S], BF16, tag="xt")
                wt_sb = xw.tile([128, ET, V3], BF16, tag="wt")
                for et in range(ET):
                    nc.sync.dma_start(xt_sb[:, et, :], xT[et * 128 : (et + 1) * 128, :])
                for et in range(ET):
                    nc.sync.dma_start(wt_sb[:, et, :], wT[et * 128 : (et + 1) * 128, :])

                # V = x @ Wv^T + bv : per s-tile [128, 256]
                for st in range(ST):
                    pt = v_ps.tile([128, 256], F32, tag="vps")
                    for et in range(ET):
                        nc.tensor.matmul(
                            pt[:],
                            xt_sb[:, et, st * 128 : (st + 1) * 128],
                            wt_sb[:, et, QK:V3],
                            start=(et == 0),
                            stop=False,
                        )
                    nc.tensor.matmul(  # + ones^T bv (bias row)
                        pt[:],
                        ones_sb[0:1, 0:128],
                        bv_sb[0:1, :],
                        start=False,
                        stop=True,
                    )
                    nc.vector.tensor_copy(v_sb[:, st, :], pt[:])

                # qkT; eo order so pair0's tiles (0: Q01, 2: K01) finish first
                for eo in (0, 2, 1, 3):
                    for sc in range(SC):
                        pt = qk_ps.tile([128, 512], F32, tag="qkps")
                        for et in range(ET):
                            nc.tensor.matmul(
                                pt[:],
                                wt_sb[:, et, eo * 128 : (eo + 1) * 128],
                                xt_sb[:, et, sc * 512 : (sc + 1) * 512],
                                start=(et == 0),
                                stop=(et == ET - 1),
                            )
                        # psum->sbuf(bf16) on DVE, adding per-partition bias
                        nc.vector.tensor_scalar_add(
                            qk_sb[:, eo, sc * 512 : (sc + 1) * 512],
                            in0=pt[:],
                            scalar1=bq_sb[:, eo : eo + 1],
                        )

            # ------- phase 4: attention per head pair ---------------------
            with (
                tc.tile_pool(name="expp", bufs=3) as expp,
                tc.tile_pool(name="sp_ps", bufs=2, space="PSUM") as sp_ps,
                tc.tile_pool(name="o_ps", bufs=1, space="PSUM") as o_ps,
            ):
                for p in range(2):  # head pairs (local heads 2p, 2p+1)
                    oT = o_ps.tile([128, S], F32, tag="ops")
                    for kt in range(KT):
                        ex = expp.tile([128, 2, S], BF16, tag="exp")
                        den = smalls.tile([128, 2, 2], F32, tag="den")
                        for half in range(2):
                            for hh in range(2):
                                sp = sp_ps.tile([128, 1024], F32, tag="sp")
                                for qc in range(2):
                                    q0 = half * 1024 + qc * 512
                                    nc.tensor.matmul(
                                        sp[:, qc * 512 : (qc + 1) * 512],
                                        qk_sb[
                                            hh * 64 : (hh + 1) * 64,
                                            2 + p,
                                            kt * 128 : (kt + 1) * 128,
                                        ],
                                        qk_sb[hh * 64 : (hh + 1) * 64, p, q0 : q0 + 512],
                                        start=True,
                                        stop=True,
                                    )
                                nc.scalar.activation(
                                    ex[:, hh, half * 1024 : (half + 1) * 1024],
                                    sp[:],
                                    AF.Exp,
                                    scale=0.125,
                                    accum_out=den[:, hh, half : half + 1],
                                )
                        dsum = smalls.tile([128, 2], F32, tag="dsum")
                        nc.vector.tensor_add(dsum[:], den[:, :, 0], den[:, :, 1])
                        rec = smalls.tile([128, 2], F32, tag="rec")
                        nc.vector.reciprocal(rec[:], dsum[:])
                        vs = smalls.tile([128, 2, DH], BF16, tag="vs")
                        for hh in range(2):
                            nc.vector.tensor_scalar_mul(
                                vs[:, hh, :],
                                in0=v_sb[:, kt, (2 * p + hh) * 64 : (2 * p + hh + 1) * 64],
                                scalar1=rec[:, hh : hh + 1],
                            )
                        for qc in range(SC):
                            for hh in range(2):
                                nc.tensor.matmul(
                                    oT[hh * 64 : (hh + 1) * 64, qc * 512 : (qc + 1) * 512],
                                    vs[:, hh, :],
                                    ex[:, hh, qc * 512 : (qc + 1) * 512],
                                    start=(kt == 0),
                                    stop=(kt == KT - 1),
                                )
                    nc.vector.tensor_copy(outT_sb[:, p, :], oT[:])

            # ------- phase 5: partial = outT^T @ WoT ----------------------
            with (
                tc.tile_pool(name="wo_out", bufs=2) as wo_out,
                tc.tile_pool(name="f_ps", bufs=2, space="PSUM") as f_ps,
            ):
                wo_sb = wo_out.tile([128, 2, E], BF16, tag="wo")
                for p in range(2):
                    nc.sync.dma_start(wo_sb[:, p, :], woT[p * 128 : (p + 1) * 128, :])
                for st in range(ST):
                    ot = wo_out.tile([128, E], F32, tag="fout")
                    for nck in range(2):
                        pt = f_ps.tile([128, 512], F32, tag="fps")
                        for p in range(2):
                            nc.tensor.matmul(
                                pt[:],
                                outT_sb[:, p, st * 128 : (st + 1) * 128],
                                wo_sb[:, p, nck * 512 : (nck + 1) * 512],
                                start=(p == 0),
                                stop=(p == 1),
                            )
                        nc.vector.tensor_copy(ot[:, nck * 512 : (nck + 1) * 512], pt[:])
                    nc.sync.dma_start(out[st * 128 : (st + 1) * 128, :], ot[:])

    nc.compile()
    return nc


def _shard_inputs(input, Wqkv, bqkv, Wo):
    """Build the 8 per-core input dicts (host-side layout/sharding)."""
    bf16 = ml_dtypes.bfloat16
    in_maps = []
    for c in range(NCORES):
        b = c // 4
        g = c % 4
        heads = range(4 * g, 4 * g + 4)
        rows = (
            [slice(64 * h, 64 * h + 64) for h in heads]
            + [slice(E + 64 * h, E + 64 * h + 64) for h in heads]
            + [slice(2 * E + 64 * h, 2 * E + 64 * h + 64) for h in heads]
        )
        W_sel = np.concatenate([Wqkv[s] for s in rows], axis=0)  # [768, 1024]
        b_sel = np.concatenate([bqkv[s] for s in rows], axis=0)  # [768]
        in_maps.append(
            {
                "xT": np.ascontiguousarray(input[b].T).astype(bf16),
                "wT": np.ascontiguousarray(W_sel.T).astype(bf16),
                "bq": np.ascontiguousarray(b_sel[:QK].reshape(4, 128).T),
                "bv": np.ascontiguousarray(b_sel[QK:V3].reshape(1, 256)).astype(bf16),
                "woT": np.ascontiguousarray(
                    Wo[:, 4 * g * DH : 4 * (g + 1) * DH].T
                ).astype(bf16),
            }
        )
    return in_maps


def kernel(input, Wqkv, bqkv, Wo, bo, _trace=False):
    global LAST_RESULTS
    input = np.asarray(input, dtype=np.float32)
    Wqkv = np.asarray(Wqkv, dtype=np.float32)
    bqkv = np.asarray(bqkv, dtype=np.float32)
    Wo = np.asarray(Wo, dtype=np.float32)
    bo = np.asarray(bo, dtype=np.float32)

    nc = build_kernel()
    in_maps = _shard_inputs(input, Wqkv, bqkv, Wo)
    kwargs = {}
    if _trace:
        kwargs = dict(trace=True, trace_cores=[0])
    res = run_bass_kernel_spmd(nc, in_maps, core_ids=list(range(NCORES)), **kwargs)
    LAST_RESULTS = res

    out = np.zeros((B, S, E), dtype=np.float32)
    for c in range(NCORES):
        out[c // 4] += res.results[c]["out"]
    out += bo
    return out


# revision 8
# speedup vs baseline: 1.4639x; 1.1701x over previous
"""Multi-head attention (softmax over the QUERY axis) on 8 TRN2 NeuronCores.

Sharding: 2 batches x 4 head-groups (4 heads each) -> 8 cores.
Each core computes, for its (batch b, heads 4g..4g+3):
    qkT = W_{q,k} @ x_b^T + b_{q,k}   [512, 2048]   (e_out on partitions)
    V   = x_b @ W_v^T + b_v           [2048, 256]
    S'  = K Q^T (scores TRANSPOSED)   [k, q] per head
    P   = exp(S'/8) with fused row-sum -> denom[k]  (softmax over q == free dim)
    outT= sum_k (V[k,:]/denom[k]) P[k,:]            [d, q] per head
    part= outT^T @ WoT_g              [2048, 1024]  (partial for this head group)
Host sums the 4 partials per batch and adds bo (the tensor-parallel epilogue).

Matmul inputs are bf16 (PSUM accumulation, softmax statistics and V'/denom
math stay fp32); host pre-casts x/W to bf16 (halves input DMA). Head pairs
share the PE array via disjoint row groups (scores: d at partitions 0/64)
and disjoint column groups (attn.V: outT partitions 0/64).

Pipelining: only the Q/K tiles for head-pair 0 are computed up front; the
remaining qkT/V work is emitted as PE "filler" groups interleaved into
pair 0's attention k-loop, so the PE never idles while ACT paces the
exp stream. attn.V accumulates in a 2-bank transient PSUM tile per
(4-ktile group, q-half) and flushes into an SBUF fp32 accumulator, keeping
total PSUM at 8 banks: S'(2x2) + attn.V(2) + qkv/final groups(2).
"""

import sys

if "/opt/trn_rl_repo" not in sys.path:
    sys.path.insert(0, "/opt/trn_rl_repo")

import numpy as np
import ml_dtypes

import concourse.bass as bass
import concourse.mybir as mybir
import concourse.tile as tile
from concourse import bacc
from concourse.bass_utils import run_bass_kernel_spmd

F32 = mybir.dt.float32
BF16 = mybir.dt.bfloat16
AF = mybir.ActivationFunctionType

B, S, E, H = 2, 2048, 1024, 16
HL = 4  # heads per core
DH = 64
QK = 512  # q+k out dims per core (2*HL*DH)
V3 = 768  # q+k+v out dims per core
NCORES = 8

ET = E // 128  # 8 e-tiles
ST = S // 128  # 16 s-tiles
SC = S // 512  # 4 s/q chunks of 512
KT = ST  # 16 k-tiles
FG = 4  # k-tiles per attn.V accumulation group

LAST_RESULTS = None


def build_kernel():
    nc = bacc.Bacc("TRN2", target_bir_lowering=False, debug=False, num_devices=NCORES)

    xT = nc.dram_tensor("xT", [E, S], BF16, kind="ExternalInput")
    wT = nc.dram_tensor("wT", [E, V3], BF16, kind="ExternalInput")
    bq = nc.dram_tensor("bq", [128, 4], F32, kind="ExternalInput")
    bv = nc.dram_tensor("bv", [1, 256], BF16, kind="ExternalInput")
    woT = nc.dram_tensor("woT", [2 * 128, E], BF16, kind="ExternalInput")
    out = nc.dram_tensor("out", [S, E], F32, kind="ExternalOutput")

    with tile.TileContext(nc) as tc:
        with (
            tc.tile_pool(name="persist", bufs=1) as persist,
            tc.tile_pool(name="smalls", bufs=3) as smalls,
            tc.tile_pool(name="expp", bufs=FG + 1) as expp,
            tc.tile_pool(name="vsp", bufs=FG + 2) as vsp,
            tc.tile_pool(name="fout", bufs=3) as foutp,
            tc.tile_pool(name="mm_ps", bufs=2, space="PSUM") as mm_ps,
            tc.tile_pool(name="sp_ps", bufs=2, space="PSUM") as sp_ps,
            tc.tile_pool(name="ot_ps", bufs=1, space="PSUM") as ot_ps,
        ):
            qk_sb = persist.tile([128, 4, S], BF16, tag="qk")
            v_sb = persist.tile([128, ST, 256], F32, tag="v")
            outT_f32 = persist.tile([128, 2, S], F32, tag="outT")
            outT_bf = persist.tile([128, 2, S], BF16, tag="outT_bf")
            bq_sb = persist.tile([128, 4], F32, tag="bq")
            bv_sb = persist.tile([1, 256], BF16, tag="bv")
            ones_sb = persist.tile([1, 512], BF16, tag="ones")
            xt_sb = persist.tile([128, ET, S], BF16, tag="xt")
            wt_sb = persist.tile([128, ET, V3], BF16, tag="wt")
            wo_sb = persist.tile([128, 2, E], BF16, tag="wo")

            nc.sync.dma_start(bq_sb[:], bq[:])
            nc.sync.dma_start(bv_sb[:], bv[:])
            nc.vector.memset(ones_sb[:], 1.0)
            for et in range(ET):
                nc.sync.dma_start(wt_sb[:, et, :], wT[et * 128 : (et + 1) * 128, :])
            for et in range(ET):
                nc.sync.dma_start(xt_sb[:, et, :], xT[et * 128 : (et + 1) * 128, :])
            for p in range(2):
                nc.sync.dma_start(wo_sb[:, p, :], woT[p * 128 : (p + 1) * 128, :])

            # ---- emitters for qkT / V accumulation groups ----------------
            def emit_qk_group(eo, sc):
                pt = mm_ps.tile([128, 512], F32, tag="mmps")
                for et in range(ET):
                    nc.tensor.matmul(
                        pt[:],
                        wt_sb[:, et, eo * 128 : (eo + 1) * 128],
                        xt_sb[:, et, sc * 512 : (sc + 1) * 512],
                        start=(et == 0),
                        stop=(et == ET - 1),
                    )
                nc.vector.tensor_scalar_add(
                    qk_sb[:, eo, sc * 512 : (sc + 1) * 512],
                    in0=pt[:],
                    scalar1=bq_sb[:, eo : eo + 1],
                )

            def emit_v_group(st):
                pt = mm_ps.tile([128, 512], F32, tag="mmps")
                for et in range(ET):
                    nc.tensor.matmul(
                        pt[:, :256],
                        xt_sb[:, et, st * 128 : (st + 1) * 128],
                        wt_sb[:, et, QK:V3],
                        start=(et == 0),
                        stop=False,
                    )
                nc.tensor.matmul(  # + ones^T bv (bias row)
                    pt[:, :256],
                    ones_sb[0:1, 0:128],
                    bv_sb[0:1, :],
                    start=False,
                    stop=True,
                )
                nc.vector.tensor_copy(v_sb[:, st, :], pt[:, :256])

            # ---- pre-attention: pair0's Q/K tiles + first V tiles --------
            for sc in range(SC):
                emit_qk_group(0, sc)  # Q heads 0,1
            for sc in range(SC):
                emit_qk_group(2, sc)  # K heads 0,1
            for st in range(FG):
                emit_v_group(st)

            fillers = (
                [lambda st=st: emit_v_group(st) for st in range(FG, ST)]
                + [lambda sc=sc: emit_qk_group(1, sc) for sc in range(SC)]
                + [lambda sc=sc: emit_qk_group(3, sc) for sc in range(SC)]
            )
            fillers.reverse()  # pop() from the front

            # ---- attention per head pair ---------------------------------
            for p in range(2):
                exs = {}
                vss = {}
                for g in range(KT // FG):
                    for kt in range(FG * g, FG * (g + 1)):
                        ex = expp.tile([128, 2, S], BF16, tag="exp")
                        exs[kt] = ex
                        den = smalls.tile([128, 2, 2], F32, tag="den")
                        for half in range(2):
                            for hh in range(2):
                                sp = sp_ps.tile([128, 1024], F32, tag="sp")
                                for qc in range(2):
                                    q0 = half * 1024 + qc * 512
                                    nc.tensor.matmul(
                                        sp[:, qc * 512 : (qc + 1) * 512],
                                        qk_sb[
                                            hh * 64 : (hh + 1) * 64,
                                            2 + p,
                                            kt * 128 : (kt + 1) * 128,
                                        ],
                                        qk_sb[hh * 64 : (hh + 1) * 64, p, q0 : q0 + 512],
                                        start=True,
                                        stop=True,
                                    )
                                nc.scalar.activation(
                                    ex[:, hh, half * 1024 : (half + 1) * 1024],
                                    sp[:],
                                    AF.Exp,
                                    scale=0.125,
                                    accum_out=den[:, hh, half : half + 1],
                                )
                        dsum = smalls.tile([128, 2], F32, tag="dsum")
                        nc.vector.tensor_add(dsum[:], den[:, :, 0], den[:, :, 1])
                        rec = smalls.tile([128, 2], F32, tag="rec")
                        nc.vector.reciprocal(rec[:], dsum[:])
                        vs = vsp.tile([128, 2, DH], BF16, tag="vs")
                        vss[kt] = vs
                        for hh in range(2):
                            nc.vector.tensor_scalar_mul(
                                vs[:, hh, :],
                                in0=v_sb[:, kt, (2 * p + hh) * 64 : (2 * p + hh + 1) * 64],
                                scalar1=rec[:, hh : hh + 1],
                            )
                        # keep the PE fed while ACT paces exp (pair 0 only)
                        if fillers:
                            fillers.pop()()
                        if fillers and kt % 2 == 1:
                            fillers.pop()()
                    # attn.V for this 4-ktile group, both heads col-paired,
                    # per q-half into a 2-bank transient accumulator
                    for half in range(2):
                        oTt = ot_ps.tile([128, 1024], F32, tag="otps")
                        for j in range(FG):
                            kt = FG * g + j
                            for hh in range(2):
                                for qc in range(2):
                                    q0 = half * 1024 + qc * 512
                                    nc.tensor.matmul(
                                        oTt[
                                            hh * 64 : (hh + 1) * 64,
                                            qc * 512 : (qc + 1) * 512,
                                        ],
                                        vss[kt][:, hh, :],
                                        exs[kt][:, hh, q0 : q0 + 512],
                                        start=(j == 0),
                                        stop=(j == FG - 1),
                                    )
                        dst = outT_f32[:, p, half * 1024 : (half + 1) * 1024]
                        if g == 0:
                            nc.vector.tensor_copy(dst, oTt[:])
                        else:
                            nc.vector.tensor_add(dst, dst, oTt[:])
                nc.vector.tensor_copy(outT_bf[:, p, :], outT_f32[:, p, :])

            # ---- final projection: partial = outT^T @ WoT ----------------
            for st in range(ST):
                ot = foutp.tile([128, E], F32, tag="fout")
                pts = [
                    mm_ps.tile([128, 512], F32, tag="mmps", name=f"fps_{st}_{i}")
                    for i in range(2)
                ]
                for p in range(2):
                    for nck in range(2):
                        nc.tensor.matmul(
                            pts[nck][:],
                            outT_bf[:, p, st * 128 : (st + 1) * 128],
                            wo_sb[:, p, nck * 512 : (nck + 1) * 512],
                            start=(p == 0),
                            stop=(p == 1),
                        )
                for nck in range(2):
                    nc.vector.tensor_copy(ot[:, nck * 512 : (nck + 1) * 512], pts[nck][:])
                nc.sync.dma_start(out[st * 128 : (st + 1) * 128, :], ot[:])

    nc.compile()
    return nc


def _shard_inputs(input, Wqkv, bqkv, Wo):
    """Build the 8 per-core input dicts (host-side layout/sharding)."""
    bf16 = ml_dtypes.bfloat16
    in_maps = []
    for c in range(NCORES):
        b = c // 4
        g = c % 4
        heads = range(4 * g, 4 * g + 4)
        rows = (
            [slice(64 * h, 64 * h + 64) for h in heads]
            + [slice(E + 64 * h, E + 64 * h + 64) for h in heads]
            + [slice(2 * E + 64 * h, 2 * E + 64 * h + 64) for h in heads]
        )
        W_sel = np.concatenate([Wqkv[s] for s in rows], axis=0)  # [768, 1024]
        b_sel = np.concatenate([bqkv[s] for s in rows], axis=0)  # [768]
        in_maps.append(
            {
                "xT": np.ascontiguousarray(input[b].T).astype(bf16),
                "wT": np.ascontiguousarray(W_sel.T).astype(bf16),
                "bq": np.ascontiguousarray(b_sel[:QK].reshape(4, 128).T),
                "bv": np.ascontiguousarray(b_sel[QK:V3].reshape(1, 256)).astype(bf16),
                "woT": np.ascontiguousarray(
                    Wo[:, 4 * g * DH : 4 * (g + 1) * DH].T
                ).astype(bf16),
            }
        )
    return in_maps


def kernel(input, Wqkv, bqkv, Wo, bo, _trace=False):
    global LAST_RESULTS
    input = np.asarray(input, dtype=np.float32)
    Wqkv = np.asarray(Wqkv, dtype=np.float32)
    bqkv = np.asarray(bqkv, dtype=np.float32)
    Wo = np.asarray(Wo, dtype=np.float32)
    bo = np.asarray(bo, dtype=np.float32)

    nc = build_kernel()
    in_maps = _shard_inputs(input, Wqkv, bqkv, Wo)
    kwargs = {}
    if _trace:
        kwargs = dict(trace=True, trace_cores=[0])
    res = run_bass_kernel_spmd(nc, in_maps, core_ids=list(range(NCORES)), **kwargs)
    LAST_RESULTS = res

    out = np.zeros((B, S, E), dtype=np.float32)
    for c in range(NCORES):
        out[c // 4] += res.results[c]["out"]
    out += bo
    return out
